# revision 15
# baseline (speedup 1.0000x reference)
"""Trainium2 Bass kernel for nn_Decoder_60198261621363.

6-layer dense transformer decoder (E=1024, H=16, FF=4096, V=32000, B=4, T=1024).

Sharding: data-parallel over tokens across 8 cores. Core c owns batch c//2,
token half c%2 (512 tokens). Weights are replicated; each layer does one pair
AllGather of that layer's K/V so the second-half cores see first-half keys.
Activations are kept feature-major (x^T: [E, tokens]) so every weight matrix
serves as the matmul stationary operand in its natural [in, out] layout.
"""
import sys

sys.path.insert(0, "/opt/trn_rl_repo")

import numpy as np

import concourse.bass as bass
import concourse.mybir as mybir
import concourse.tile as tile
from concourse import bacc
from concourse.bass_utils import run_bass_kernel_spmd
from concourse.masks import make_identity

AF = mybir.ActivationFunctionType
F32 = mybir.dt.float32
I32 = mybir.dt.int32

V, E, L, H, B, T = 32000, 1024, 6, 16, 4, 1024
HS = E // H          # 64
FF = 4 * E           # 4096
NC = 8               # cores
RT = 512             # tokens per core
KT = E // 128        # 8  E-tiles
FT = FF // 128       # 32 ff-tiles
TT = RT // 128       # 4  token-tiles per core
VCH = 500            # lm-head column chunk
NVC = V // VCH       # 64 chunks
SCALE = float(E) ** -0.5

_cache = {}


def _build(n_layers=L):
    nc = bacc.Bacc("TRN2", target_bir_lowering=False, debug=False, num_devices=NC)

    tok_emb = nc.dram_tensor("tok_emb", [V, E], F32, kind="ExternalInput")
    tok_idx = nc.dram_tensor("tok_idx", [128, TT], I32, kind="ExternalInput")
    posT = nc.dram_tensor("posT", [E, RT], F32, kind="ExternalInput")
    maskT = nc.dram_tensor("maskT", [E, RT], F32, kind="ExternalInput")
    Wq = nc.dram_tensor("Wq", [n_layers, E, E], F32, kind="ExternalInput")
    Wk = nc.dram_tensor("Wk", [n_layers, E, E], F32, kind="ExternalInput")
    Wv = nc.dram_tensor("Wv", [n_layers, E, E], F32, kind="ExternalInput")
    Wo = nc.dram_tensor("Wo", [n_layers, E, E], F32, kind="ExternalInput")
    W1 = nc.dram_tensor("W1", [n_layers, E, FF], F32, kind="ExternalInput")
    W2 = nc.dram_tensor("W2", [n_layers, FF, E], F32, kind="ExternalInput")
    # packed per-partition params: [n_layers, 128, ntiles]
    ln1g = nc.dram_tensor("ln1g", [n_layers, 128, KT], F32, kind="ExternalInput")
    ln1b = nc.dram_tensor("ln1b", [n_layers, 128, KT], F32, kind="ExternalInput")
    ln2g = nc.dram_tensor("ln2g", [n_layers, 128, KT], F32, kind="ExternalInput")
    ln2b = nc.dram_tensor("ln2b", [n_layers, 128, KT], F32, kind="ExternalInput")
    b1d = nc.dram_tensor("b1", [n_layers, 128, FT], F32, kind="ExternalInput")
    b2d = nc.dram_tensor("b2", [n_layers, 128, KT], F32, kind="ExternalInput")
    lmhw = nc.dram_tensor("lmhw", [E, V], F32, kind="ExternalInput")
    lmhb = nc.dram_tensor("lmhb", [128, V], F32, kind="ExternalInput")
    logits = nc.dram_tensor("logits", [RT, V], F32, kind="ExternalOutput")

    rgroups = [[0, 1], [2, 3], [4, 5], [6, 7]]

    with tile.TileContext(nc) as tc:
        with (
            tc.tile_pool(name="persist", bufs=1) as pp,
            tc.tile_pool(name="sb", bufs=2) as sb,
            tc.tile_pool(name="wstream", bufs=2) as wp,
            tc.tile_pool(name="psd", bufs=3, space="PSUM") as psd,   # dense mms
            tc.tile_pool(name="pss", bufs=2, space="PSUM") as pss,   # scores
            tc.tile_pool(name="psa", bufs=1, space="PSUM") as psa,   # attn out
            tc.tile_pool(name="pst", bufs=1, space="PSUM") as pst,   # ln stats
            tc.tile_pool(name="dram", bufs=2, space="DRAM") as dram,
        ):
            # ---- persistent tiles
            x = pp.tile([128, KT, RT], F32)        # residual stream x^T
            mask = pp.tile([128, KT, RT], F32)     # causal mask (key-tile, query)
            ones = pp.tile([128, 128], F32)
            ident = pp.tile([128, 128], F32)
            epsb = pp.tile([128, 1], F32)
            nc.vector.memset(ones[:], 1.0)
            nc.vector.memset(epsb[:], 1e-5)
            make_identity(nc, ident[:])
            nc.sync.dma_start(mask[:], maskT.ap().rearrange("(k p) t -> p k t", p=128))

            # ---- embedding gather + transpose + pos add
            idx = pp.tile([128, TT], I32)
            nc.sync.dma_start(idx[:], tok_idx[:])
            for pc in range(2):
                pos_sb = wp.tile([128, KT, 256], F32, tag="wstream")
                nc.sync.dma_start(
                    pos_sb[:],
                    posT.ap().rearrange("(k p) t -> p k t", p=128)
                    [:, :, pc * 256:(pc + 1) * 256])
                for gg in range(2):
                    g = pc * 2 + gg
                    emb = sb.tile([128, E], F32, tag="embg", bufs=1)
                    nc.gpsimd.indirect_dma_start(
                        out=emb[:], out_offset=None, in_=tok_emb[:],
                        in_offset=bass.IndirectOffsetOnAxis(
                            ap=idx[:, g:g + 1], axis=0),
                    )
                    for kt in range(KT):
                        tp = psd.tile([128, 128], F32, space="PSUM", tag="psd")
                        nc.tensor.transpose(tp[:], emb[:, kt * 128:(kt + 1) * 128],
                                            ident[:])
                        nc.vector.tensor_add(
                            out=x[:, kt, g * 128:(g + 1) * 128],
                            in0=tp[:],
                            in1=pos_sb[:, kt, gg * 128:(gg + 1) * 128])

            def layer_norm(gd, bd, li):
                """h = LN(x) * g + b, returns h tile [128, KT, RT]."""
                gt = sb.tile([128, KT], F32, tag="lnp_g")
                bt = sb.tile([128, KT], F32, tag="lnp_b")
                nc.sync.dma_start(gt[:], gd[li])
                nc.sync.dma_start(bt[:], bd[li])
                sum_ps = pst.tile([128, RT], F32, space="PSUM", tag="pst")
                sq_ps = pst.tile([128, RT], F32, space="PSUM", tag="pst2")
                for kt in range(KT):
                    sq = sb.tile([128, RT], F32, tag="lnsq")
                    nc.scalar.activation(sq[:], x[:, kt, :], AF.Square)
                    nc.tensor.matmul(sum_ps[:], ones[:], x[:, kt, :],
                                     start=(kt == 0), stop=(kt == KT - 1))
                    nc.tensor.matmul(sq_ps[:], ones[:], sq[:],
                                     start=(kt == 0), stop=(kt == KT - 1))
                m = sb.tile([128, RT], F32, tag="ln_m", bufs=1)
                nc.vector.tensor_scalar_mul(m[:], sum_ps[:], 1.0 / E)
                mm = sb.tile([128, RT], F32, tag="ln_t")
                nc.vector.tensor_mul(mm[:], m[:], m[:])
                var = sb.tile([128, RT], F32, tag="ln_t")
                nc.vector.scalar_tensor_tensor(
                    var[:], sq_ps[:], 1.0 / E, mm[:],
                    mybir.AluOpType.mult, mybir.AluOpType.subtract)
                std = sb.tile([128, RT], F32, tag="ln_t")
                nc.scalar.activation(std[:], var[:], AF.Sqrt, bias=epsb[:, 0:1])
                r = sb.tile([128, RT], F32, tag="ln_r", bufs=1)
                nc.vector.reciprocal(r[:], std[:])
                h = sb.tile([128, KT, RT], F32, tag="h", bufs=1)
                for kt in range(KT):
                    d = sb.tile([128, RT], F32, tag="ln_d")
                    nc.vector.tensor_sub(d[:], x[:, kt, :], m[:])
                    nc.vector.tensor_mul(d[:], d[:], r[:])
                    nc.vector.tensor_scalar(
                        h[:, kt, :], d[:], gt[:, kt:kt + 1], bt[:, kt:kt + 1],
                        mybir.AluOpType.mult, mybir.AluOpType.add)
                return h

            for li in range(n_layers):
                # ================= attention =================
                h = layer_norm(ln1g, ln1b, li)

                agin = dram.tile([2, E * RT], F32, tag="agin")
                agout = dram.tile([2, 2, E * RT], F32, tag="agout")
                # shaped views: K^T is [E, RT] feature-major, V is [RT, E] row-major
                kin = agin[0].rearrange("(f t) -> f t", t=RT)
                vin = agin[1].rearrange("(t f) -> t f", f=E)
                kout = [agout[s, 0].rearrange("(f t) -> f t", t=RT)
                        for s in range(2)]
                vout = [agout[s, 1].rearrange("(t f) -> t f", f=E)
                        for s in range(2)]

                # Q^T (keep), K^T (to agin[0]), V row-major (to agin[1])
                qT = sb.tile([128, KT, RT], F32, tag="qT", bufs=1)
                for w_dram, dst in ((Wq, "q"), (Wk, "k")):
                    for ch in range(4):
                        wt = wp.tile([128, KT, 256], F32, tag="wstream")
                        nc.sync.dma_start(
                            wt[:],
                            w_dram[li].rearrange("(k p) m -> p k m", p=128)
                            [:, :, ch * 256:(ch + 1) * 256])
                        for mt in range(2):
                            ps = psd.tile([128, RT], F32, space="PSUM", tag="psd")
                            for kt in range(KT):
                                nc.tensor.matmul(
                                    ps[:], wt[:, kt, mt * 128:(mt + 1) * 128],
                                    h[:, kt, :],
                                    start=(kt == 0), stop=(kt == KT - 1))
                            ft = ch * 2 + mt
                            if dst == "q":
                                nc.vector.tensor_copy(qT[:, ft, :], ps[:])
                            else:
                                kv = sb.tile([128, RT], F32, tag="kvb")
                                nc.vector.tensor_copy(kv[:], ps[:])
                                nc.sync.dma_start(
                                    kin[ft * 128:(ft + 1) * 128, :], kv[:])
                # V row-major: out[tok, feat]
                for ch in range(4):
                    wt = wp.tile([128, KT, 256], F32, tag="wstream")
                    nc.sync.dma_start(
                        wt[:],
                        Wv[li].rearrange("(k p) m -> p k m", p=128)
                        [:, :, ch * 256:(ch + 1) * 256])
                    for tt in range(TT):
                        ps = psd.tile([128, 256], F32, space="PSUM", tag="psd")
                        for kt in range(KT):
                            nc.tensor.matmul(
                                ps[:], h[:, kt, tt * 128:(tt + 1) * 128],
                                wt[:, kt, :],
                                start=(kt == 0), stop=(kt == KT - 1))
                        kv = sb.tile([128, 256], F32, tag="kvb")
                        nc.vector.tensor_copy(kv[:], ps[:])
                        nc.sync.dma_start(
                            vin[tt * 128:(tt + 1) * 128,
                                ch * 256:(ch + 1) * 256], kv[:])

                nc.gpsimd.collective_compute(
                    "AllGather", mybir.AluOpType.bypass,
                    replica_groups=rgroups,
                    ins=[agin[:].opt()], outs=[agout[:].opt()],
                )

                aT = sb.tile([128, KT, RT], F32, tag="aT", bufs=1)
                for hd in range(H):
                    hp, lo = hd // 2, (hd % 2) * 64
                    kp = sb.tile([128, 2, RT], F32, tag="kpair")
                    for s in range(2):
                        nc.sync.dma_start(
                            kp[:, s, :], kout[s][hp * 128:(hp + 1) * 128, :])
                    vo = sb.tile([128, KT, HS + 1], F32, tag="vones")
                    nc.vector.memset(vo[:, :, HS:HS + 1], 1.0)
                    for s in range(2):
                        for ktl in range(TT):
                            kt = s * TT + ktl
                            nc.sync.dma_start(
                                vo[:, kt, 0:HS],
                                vout[s][ktl * 128:(ktl + 1) * 128,
                                        hd * HS:(hd + 1) * HS])
                    av = psa.tile([HS + 1, RT], F32, space="PSUM", tag="psa")
                    for kt in range(KT):
                        s, ktl = kt // TT, kt % TT
                        sc = pss.tile([128, RT], F32, space="PSUM", tag="pss")
                        nc.tensor.matmul(
                            sc[:],
                            kp[lo:lo + 64, kt // 4, (kt % 4) * 128:(kt % 4 + 1) * 128],
                            qT[lo:lo + 64, hp, :],
                            start=True, stop=True)
                        e = sb.tile([128, RT], F32, tag="esc")
                        nc.scalar.activation(e[:], sc[:], AF.Exp, scale=SCALE)
                        nc.vector.tensor_mul(e[:], e[:], mask[:, kt, :])
                        nc.tensor.matmul(av[:], vo[:, kt, :], e[:],
                                         start=(kt == 0), stop=(kt == KT - 1))
                    r1 = sb.tile([1, RT], F32, tag="rd")
                    nc.vector.reciprocal(r1[:], av[HS:HS + 1, :])
                    rb = sb.tile([64, RT], F32, tag="rb")
                    nc.gpsimd.partition_broadcast(rb[:], r1[0:1, :])
                    nc.vector.tensor_mul(aT[lo:lo + 64, hp, :], av[0:HS, :], rb[:])

                # ---- Wo + residual
                for ch in range(4):
                    wt = wp.tile([128, KT, 256], F32, tag="wstream")
                    nc.sync.dma_start(
                        wt[:],
                        Wo[li].rearrange("(k p) m -> p k m", p=128)
                        [:, :, ch * 256:(ch + 1) * 256])
                    for mt in range(2):
                        ps = psd.tile([128, RT], F32, space="PSUM", tag="psd")
                        for kt in range(KT):
                            nc.tensor.matmul(
                                ps[:], wt[:, kt, mt * 128:(mt + 1) * 128],
                                aT[:, kt, :],
                                start=(kt == 0), stop=(kt == KT - 1))
                        ft = ch * 2 + mt
                        nc.vector.tensor_add(x[:, ft, :], x[:, ft, :], ps[:])

                # ================= FFN =================
                h2 = layer_norm(ln2g, ln2b, li)
                b1t = sb.tile([128, FT], F32, tag="b1t")
                nc.sync.dma_start(b1t[:], b1d[li])
                b2t = sb.tile([128, KT], F32, tag="b2t")
                nc.sync.dma_start(b2t[:], b2d[li])
                for ffc in range(4):
                    up = sb.tile([128, KT, RT], F32, tag="up", bufs=1)
                    for ch in range(4):
                        wt = wp.tile([128, KT, 256], F32, tag="wstream")
                        nc.sync.dma_start(
                            wt[:],
                            W1[li].rearrange("(k p) m -> p k m", p=128)
                            [:, :, ffc * 1024 + ch * 256: ffc * 1024 + (ch + 1) * 256])
                        for mt in range(2):
                            ps = psd.tile([128, RT], F32, space="PSUM", tag="psd")
                            for kt in range(KT):
                                nc.tensor.matmul(
                                    ps[:], wt[:, kt, mt * 128:(mt + 1) * 128],
                                    h2[:, kt, :],
                                    start=(kt == 0), stop=(kt == KT - 1))
                            uft = ch * 2 + mt
                            gft = ffc * 8 + uft
                            nc.scalar.activation(up[:, uft, :], ps[:], AF.Relu,
                                                 bias=b1t[:, gft:gft + 1])
                    for ch in range(4):
                        wt = wp.tile([128, KT, 256], F32, tag="wstream")
                        nc.sync.dma_start(
                            wt[:],
                            W2[li].rearrange("(k p) m -> p k m", p=128)
                            [:, ffc * 8:(ffc + 1) * 8, ch * 256:(ch + 1) * 256])
                        for mt in range(2):
                            ps = psd.tile([128, RT], F32, space="PSUM", tag="psd")
                            for kt in range(KT):
                                nc.tensor.matmul(
                                    ps[:], wt[:, kt, mt * 128:(mt + 1) * 128],
                                    up[:, kt, :],
                                    start=(kt == 0), stop=(kt == KT - 1))
                            ft = ch * 2 + mt
                            if ffc == 3:
                                nc.vector.scalar_tensor_tensor(
                                    x[:, ft, :], ps[:], b2t[:, ft:ft + 1],
                                    x[:, ft, :],
                                    mybir.AluOpType.add, mybir.AluOpType.add)
                            else:
                                nc.vector.tensor_add(x[:, ft, :], x[:, ft, :],
                                                     ps[:])

            # ================= LM head =================
            for vc in range(NVC):
                wt = wp.tile([128, KT, VCH], F32, tag="lmhs", bufs=2)
                nc.sync.dma_start(
                    wt[:],
                    lmhw.ap().rearrange("(k p) v -> p k v", p=128)
                    [:, :, vc * VCH:(vc + 1) * VCH])
                bb = sb.tile([128, VCH], F32, tag="bb")
                nc.sync.dma_start(bb[:], lmhb[:, vc * VCH:(vc + 1) * VCH])
                for tt in range(TT):
                    ps = psd.tile([128, VCH], F32, space="PSUM", tag="psd")
                    for kt in range(KT):
                        nc.tensor.matmul(
                            ps[:], x[:, kt, tt * 128:(tt + 1) * 128],
                            wt[:, kt, :],
                            start=(kt == 0), stop=(kt == KT - 1))
                    lg = sb.tile([128, VCH], F32, tag="lg", bufs=2)
                    nc.vector.tensor_add(lg[:], ps[:], bb[:])
                    nc.sync.dma_start(
                        logits[tt * 128:(tt + 1) * 128, vc * VCH:(vc + 1) * VCH],
                        lg[:])
    nc.compile()
    return nc


def _prepare(inputs, n_layers=L):
    """Build the 8 per-core input maps from full inputs."""
    f = lambda a: np.ascontiguousarray(np.asarray(a), dtype=np.float32)
    tokens = np.asarray(inputs["tokens"]).astype(np.int32)
    tok_emb = f(inputs["tok_emb"])
    pos_emb = f(inputs["pos_emb"])
    lnpack = lambda a: np.ascontiguousarray(
        f(a)[:n_layers].reshape(n_layers, -1, 128).transpose(0, 2, 1))
    common = {
        "tok_emb": tok_emb,
        "Wq": f(inputs["Wq"])[:n_layers], "Wk": f(inputs["Wk"])[:n_layers],
        "Wv": f(inputs["Wv"])[:n_layers], "Wo": f(inputs["Wo"])[:n_layers],
        "W1": f(inputs["W1"])[:n_layers], "W2": f(inputs["W2"])[:n_layers],
        "ln1g": lnpack(inputs["ln1_g"]), "ln1b": lnpack(inputs["ln1_b"]),
        "ln2g": lnpack(inputs["ln2_g"]), "ln2b": lnpack(inputs["ln2_b"]),
        "b1": lnpack(inputs["b1"]), "b2": lnpack(inputs["b2"]),
        "lmhw": f(inputs["lmh_w"]),
        "lmhb": np.ascontiguousarray(
            np.broadcast_to(f(inputs["lmh_b"])[None, :], (128, V))),
    }
    in_maps = []
    for c in range(NC):
        b, hf = c // 2, c % 2
        t0 = hf * RT
        toks = tokens[b, t0:t0 + RT]
        tok_idx = np.ascontiguousarray(toks.reshape(TT, 128).T)
        posT = np.ascontiguousarray(pos_emb[t0:t0 + RT].T)
        k = np.arange(T)[:, None]
        q = np.arange(t0, t0 + RT)[None, :]
        maskT = (k <= q).astype(np.float32)
        in_maps.append(dict(common, tok_idx=tok_idx, posT=posT, maskT=maskT))
    return in_maps


def kernel(**inputs):
    key = "nc"
    if key not in _cache:
        _cache[key] = _build()
    nc = _cache[key]
    in_maps = _prepare(inputs)
    res = run_bass_kernel_spmd(nc, in_maps, core_ids=list(range(NC)))
    out = np.empty((B, T, V), np.float32)
    for c in range(NC):
        b, hf = c // 2, c % 2
        out[b, hf * RT:(hf + 1) * RT] = res.results[c]["logits"]
    return out
